# revision 34
# baseline (speedup 1.0000x reference)
"""GroupWiseTemporalAttention Trainium2 kernel.

Math: in the reference, SDPA runs with seq-len L=S=1 per channel-group, so
softmax over the single key is identically 1 and the attention output equals
v = (x+pe)_group @ v_w.T + v_b.  The whole module therefore folds into one
affine map:

    out = x_flat @ W_eff + b_eff
    W_eff = kron(I_192, v_w.T) @ proj_w.T            # [768, 768]
    b_eff = pe@W_eff + tile(v_b,192)@proj_w.T + proj_b

which we run as a data-parallel GEMM over 8 NeuronCores (6272 rows each).
The per-core kernel streams pre-transposed x^T tiles as the stationary
matmul operand so output lands in natural [tokens, channels] layout.

Head schedule: every HWDGE dma_start costs ~0.65us of sequencer issue
time and ~2us of completion-receipt latency, so the critical head data
(weights + first token block) is fused host-side into one
[128, 6, 1280] tensor and fetched with per-kc chunk DMAs alternating
across both HWDGE rings.  Block 0 runs kc-outer so matmul pass k only
needs head chunk k -- the PE rides the chunk arrival curve instead of
waiting for the full weight matrix.  ~16 pre-warm matmuls bridge the
PE from the end of the NEFF preamble to the first chunk's arrival so
the HAM clock-gate un-throttles to 2.4GHz as the real stream begins.
Later x blocks ramp [1, 2, 4 x 10, 1, 1] behind the bf16 bias so every
consumer deadline trails its DMA arrival; outputs (bf16, upcast on the
host) ride the scalar ring, with the final tile's store split across
both rings to pay its HBM receipt in parallel on the tail.
"""

import os

import numpy as np
import ml_dtypes

import concourse.bass as bass
import concourse.mybir as mybir
import concourse.tile as tile
from concourse import bacc
from concourse.bass_utils import run_bass_kernel_spmd

P = 128
C = 768
KC = C // P            # 6 contraction chunks
N_CORES = 8
B, H, W = 16, 56, 56
ROWS = B * H * W       # 50176
RPC = ROWS // N_CORES  # 6272 rows per core
TT = RPC // P          # 49 token tiles per core
TBLK = 4               # token tiles per input DMA block (512 tokens)
HBLK = 4               # token tiles in the fused head block
HSUB = 4               # tiles per kc-outer sub-block (psum capacity)
HW_ = C + HBLK * P     # head chunk cols per kc: 768 w + 1024 x0
N_WARM = 16            # PE pre-warm matmuls issued during the DMA head
OUT_BF16 = os.environ.get("GWTA_OUT", "bf16") == "bf16"

# Internal matmul dtype: bf16 halves input DMA and streams 1 col/cycle.
VARIANT = os.environ.get("GWTA_VARIANT", "bf16")

LAST_STATS: dict = {}

_IN_DT = {
    "bf16": mybir.dt.bfloat16,
    "fp32r": mybir.dt.float32r,
    "fp32": mybir.dt.float32,
}


def _build_nc(variant: str) -> bass.Bass:
    in_dt = _IN_DT[variant]
    out_dt = mybir.dt.bfloat16 if OUT_BF16 else mybir.dt.float32
    nc = bacc.Bacc(None, target_bir_lowering=False)
    hx = nc.declare_dram_parameter("hx", [P, KC, HW_], in_dt, isOutput=False)
    xT = nc.declare_dram_parameter("xT", [C, RPC], in_dt, isOutput=False)
    b = nc.declare_dram_parameter("b", [P, C], mybir.dt.bfloat16, isOutput=False)
    out = nc.declare_dram_parameter("out", [RPC, C], out_dt, isOutput=True)

    with tile.TileContext(nc) as tc:
        with (
            tc.tile_pool(name="const", bufs=1) as const,
            tc.tile_pool(name="xp", bufs=3) as xp,
            tc.tile_pool(name="op", bufs=8) as op,
            tc.tile_pool(name="pp", bufs=1, space="PSUM") as pp,
        ):
            # PE pre-warm: matmuls on zeroed SBUF keep the PE busy from the
            # end of the NEFF preamble (~6.5us) until the first head chunk
            # lands (~9.7us), so the HAM activity window accumulates
            # continuous busy time and un-throttles to 2.4GHz shortly after
            # the real stream begins.  The memset is small so the first
            # warm matmul issues as early as possible.
            g_rhs = const.tile([P, 256], in_dt)
            nc.vector.memset(g_rhs[:], 0.0)
            warm = pp.tile([P, C], mybir.dt.float32, tag="pt3")
            for _ in range(N_WARM):
                nc.tensor.matmul(
                    warm[:, 0:256], g_rhs[:, 0:P], g_rhs[:],
                    start=True, stop=True,
                )

            xTr = xT.rearrange("(kc p) t -> p kc t", p=P)

            # Head chunks: [w_kc | x0_kc] fused, one DMA per kc on the sync
            # ring.  Pass kc of block 0 needs only chunk kc.  Chunk 0 is
            # split so the very first matmul is gated by just w0 + one x
            # tile (229KB) instead of the full chunk.
            # Chunks alternate between the two HWDGE rings: a single ring
            # sustains only ~280GB/s because each DMA's completion receipt
            # (~1-2us) bubbles its queue; the SDMA engines round-robin
            # across queues at packet granularity, so two rings pipeline
            # one chunk's receipt under the other's transfer.
            ht = const.tile([P, KC, HW_], in_dt, name="ht")
            nc.sync.dma_start(out=ht[:, 0, 0 : C + P], in_=hx[:, 0, 0 : C + P])
            nc.scalar.dma_start(out=ht[:, 0, C + P :], in_=hx[:, 0, C + P :])
            for kc in range(1, KC):
                eng = nc.sync if kc % 2 else nc.scalar
                eng.dma_start(out=ht[:, kc, :], in_=hx[:, kc, :])
            # Ramp: tiny blocks right after the head so each deadline
            # trails its DMA arrival, a 1-tile final block for a short tail.
            blocks = [1, 2] + [TBLK] * 10 + [1, 1]
            assert HBLK + sum(blocks) == TT

            xbts: dict = {}

            def issue_xb(j: int) -> None:
                nbj = blocks[j]
                t0j = HBLK + sum(blocks[:j])
                xt = xp.tile([P, KC, TBLK * P], in_dt, tag="xb", name="xb")
                nc.sync.dma_start(
                    out=xt[:, :, : nbj * P],
                    in_=xTr[:, :, t0j * P : (t0j + nbj) * P],
                )
                xbts[j] = xt

            # The bias (bf16) gates block 0's psum evictions, whose tags
            # block 1's later tiles reuse.  It follows the head chunks on
            # the scalar ring, well before the first output (~20us).
            bt = const.tile([P, C], mybir.dt.bfloat16)
            nc.scalar.dma_start(out=bt[:], in_=b[:])
            issue_xb(0)
            issue_xb(1)

            def wslice(kc, half):
                return (
                    ht[:, kc, 0:512] if half == 0 else ht[:, kc, 512:C]
                )

            def evict_last(pt) -> None:
                # Final tile: per-bank TT halves, each store issued the
                # moment its half is ready, split across both HWDGE rings.
                # The kc=5 matmul emission order below makes the [512:C]
                # bank finish FIRST, so its TT and store run while the PE
                # streams the last 512-col matmul.
                g = TT - 1
                ot = op.tile([P, C], out_dt, tag="ot")
                nc.vector.tensor_add(
                    out=ot[:, 512:C], in0=pt[:, 512:C], in1=bt[:, 512:C]
                )
                nc.sync.dma_start(
                    out=out[g * P : (g + 1) * P, 512:C], in_=ot[:, 512:C]
                )
                nc.vector.tensor_add(
                    out=ot[:, 0:512], in0=pt[:, 0:512], in1=bt[:, 0:512]
                )
                nc.scalar.dma_start(
                    out=out[g * P : (g + 1) * P, 0:512], in_=ot[:, 0:512]
                )

            def evict(g: int, pt) -> None:
                ot = op.tile([P, C], out_dt, tag="ot")
                # One fused TT per tile: DVE reads may span psum banks
                # (only matmul WRITES are bank-limited), it is ~25% faster
                # than a 512+256 pair, and it keeps each eviction atomic --
                # the scheduler otherwise interleaves pair halves across
                # tiles, delaying the psum-tag release the next block waits
                # on.
                nc.vector.tensor_add(
                    out=ot[:], in0=pt[:], in1=bt[:]
                )
                nc.scalar.dma_start(
                    out=out[g * P : (g + 1) * P, :], in_=ot[:]
                )

            # Block 0: two kc-outer sub-blocks of HSUB tiles -- pass kc
            # only needs head chunk kc, so the PE rides the chunk arrival
            # curve from the first one.  Each kc=5 pass runs s-ascending,
            # so psum tag g%4 is cleared in the order the next sub-block
            # reuses it.
            for sub in range(HBLK // HSUB):
                pts = [
                    pp.tile(
                        [P, C], mybir.dt.float32,
                        tag=f"pt{(sub * HSUB + s) % 4}",
                        name=f"pt{(sub * HSUB + s) % 4}",
                    )
                    for s in range(HSUB)
                ]
                for kc in range(KC):
                    for s in range(HSUB):
                        g = sub * HSUB + s
                        lhsT = ht[:, kc, C + g * P : C + (g + 1) * P]
                        nc.tensor.matmul(
                            pts[s][:, 0:512], lhsT, wslice(kc, 0),
                            start=(kc == 0), stop=(kc == KC - 1),
                        )
                        nc.tensor.matmul(
                            pts[s][:, 512:C], lhsT, wslice(kc, 1),
                            start=(kc == 0), stop=(kc == KC - 1),
                        )
                for s in range(HSUB):
                    evict(sub * HSUB + s, pts[s])

            # Blocks 1..: s-outer, one 4-tile input DMA each, prefetch
            # distance 2.
            g0 = HBLK
            for bi, nb in enumerate(blocks):
                if bi + 2 < len(blocks):
                    issue_xb(bi + 2)
                xt = xbts.pop(bi)
                # Natural %4 rotation: tile g reuses the tag freed by
                # tile g-4, whose eviction is the EARLIEST completed of the
                # previous four (block 0's kc-outer evictions release in
                # s-ascending order).
                pts = [
                    pp.tile(
                        [P, C], mybir.dt.float32,
                        tag=f"pt{(g0 + s) % 4}", name=f"pt{(g0 + s) % 4}",
                    )
                    for s in range(nb)
                ]
                for s in range(nb):
                    for kc in range(KC):
                        lhsT = xt[:, kc, s * P : (s + 1) * P]
                        halves = (0, 1)
                        if g0 + s == TT - 1 and kc == KC - 1:
                            # Last matmul pair of the stream: finish the
                            # [512:C] bank first so its eviction + store
                            # overlap the final 512-col matmul.
                            halves = (1, 0)
                        for h in halves:
                            nc.tensor.matmul(
                                pts[s][:, 0:512] if h == 0
                                else pts[s][:, 512:C],
                                lhsT, wslice(kc, h),
                                start=(kc == 0), stop=(kc == KC - 1),
                            )
                for s in range(nb):
                    if g0 + s == TT - 1:
                        evict_last(pts[s])
                    else:
                        evict(g0 + s, pts[s])
                g0 += nb
    nc.compile()
    return nc


def _fold_weights(qkv_w, qkv_b, proj_w, proj_b, pe):
    v_w = qkv_w[2 * 4 : 3 * 4].astype(np.float64)   # [4, 4]
    v_b = qkv_b[2 * 4 : 3 * 4].astype(np.float64)   # [4]
    bd = np.kron(np.eye(C // 4), v_w.T)             # y_flat @ bd == groupwise v
    w_eff = bd @ proj_w.astype(np.float64).T        # [768, 768]
    b_eff = (
        np.tile(v_b, C // 4) @ proj_w.astype(np.float64).T
        + proj_b.astype(np.float64)
        + pe[:C].astype(np.float64) @ w_eff
    )
    return w_eff, b_eff


def _enable_tracing_shims():
    """Dev-only (GWTA_TRACE=1): restore the NTFF profile hook that this
    image's `antenv` is missing, and keep trace artifacts local instead of
    uploading.  Never active when the kernel is called normally."""
    import sys
    import types

    try:
        from antenv import axon_hooks  # noqa: F401
    except ImportError:
        import antenv
        from trn_agent_boot.trn_boot import _ntff_profile_via_ctypes

        mod = types.ModuleType("antenv.axon_hooks")
        mod._hook = _ntff_profile_via_ctypes("/opt/axon/libaxon_pjrt.so")
        mod.get_axon_ntff_profile_hook = lambda: mod._hook
        mod.set_axon_ntff_profile_hook = lambda h: setattr(mod, "_hook", h)
        sys.modules["antenv.axon_hooks"] = mod
        antenv.axon_hooks = mod

    import concourse.bass_utils as bu

    bu.upload_artifacts = lambda tmpdir: f"local:{tmpdir}"


def kernel(x, qkv_w, qkv_b, proj_w, proj_b, pe):
    x = np.asarray(x, np.float32)
    w_eff, b_eff = _fold_weights(
        np.asarray(qkv_w), np.asarray(qkv_b),
        np.asarray(proj_w), np.asarray(proj_b), np.asarray(pe),
    )

    variant = VARIANT
    if variant == "bf16":
        cast = lambda a: np.ascontiguousarray(a, dtype=ml_dtypes.bfloat16)
    else:
        cast = lambda a: np.ascontiguousarray(a, dtype=np.float32)

    w_dev = cast(w_eff)                       # [768, 768]
    w_chunks = np.asarray(w_dev).reshape(KC, P, C)  # [kc, p, c]
    b_dev = np.broadcast_to(
        b_eff.astype(np.float32).astype(ml_dtypes.bfloat16), (P, C)
    ).copy()

    x_flat = x.reshape(ROWS, C)
    in_maps = []
    for c in range(N_CORES):
        xc = x_flat[c * RPC : (c + 1) * RPC]
        xt = cast(xc.T)                       # [768, 6272]
        xt_np = np.asarray(xt)
        # Fused head: [p, kc, 768 w | 512 x0]
        hx = np.empty((P, KC, HW_), dtype=xt_np.dtype)
        hx[:, :, :C] = w_chunks.transpose(1, 0, 2)
        hx[:, :, C:] = (
            xt_np[:, : HBLK * P].reshape(KC, P, HBLK * P).transpose(1, 0, 2)
        )
        in_maps.append({"hx": hx, "xT": xt_np, "b": b_dev})

    nc = _build_nc(variant)
    trace = bool(int(os.environ.get("GWTA_TRACE", "0")))
    kw = {}
    if trace:
        _enable_tracing_shims()
        kw["tmpdir"] = os.environ.get("GWTA_TRACE_DIR") or None
    r = run_bass_kernel_spmd(nc, in_maps, list(range(N_CORES)), trace=trace, **kw)

    LAST_STATS.clear()
    LAST_STATS.update(
        exec_time_ns=r.exec_time_ns,
        mean_exec_time_ns=r.mean_exec_time_ns,
        variant=variant,
    )

    out = np.empty((ROWS, C), np.float32)
    for c in range(N_CORES):
        out[c * RPC : (c + 1) * RPC] = np.asarray(
            r.results[c]["out"]
        ).astype(np.float32)
    return out.reshape(B, H, W, C)


# revision 35
# speedup vs baseline: 1.0052x; 1.0052x over previous
"""GroupWiseTemporalAttention Trainium2 kernel.

Math: in the reference, SDPA runs with seq-len L=S=1 per channel-group, so
softmax over the single key is identically 1 and the attention output equals
v = (x+pe)_group @ v_w.T + v_b.  The whole module therefore folds into one
affine map:

    out = x_flat @ W_eff + b_eff
    W_eff = kron(I_192, v_w.T) @ proj_w.T            # [768, 768]
    b_eff = pe@W_eff + tile(v_b,192)@proj_w.T + proj_b

which we run as a data-parallel GEMM over 8 NeuronCores (6272 rows each).
The per-core kernel streams pre-transposed x^T tiles as the stationary
matmul operand so output lands in natural [tokens, channels] layout.

Head schedule: every HWDGE dma_start costs ~0.65us of sequencer issue
time and ~2us of completion-receipt latency, so the critical head data
(weights + first token block) is fused host-side into one
[128, 6, 1280] tensor and fetched with per-kc chunk DMAs alternating
across both HWDGE rings.  Block 0 runs kc-outer so matmul pass k only
needs head chunk k -- the PE rides the chunk arrival curve instead of
waiting for the full weight matrix.  ~16 pre-warm matmuls bridge the
PE from the end of the NEFF preamble to the first chunk's arrival so
the HAM clock-gate un-throttles to 2.4GHz as the real stream begins.
Later x blocks ramp [1, 2, 4 x 10, 1, 1] behind the bf16 bias so every
consumer deadline trails its DMA arrival; outputs (bf16, upcast on the
host) ride the scalar ring, with the final tile's store split across
both rings to pay its HBM receipt in parallel on the tail.
"""

import os

import numpy as np
import ml_dtypes

import concourse.bass as bass
import concourse.mybir as mybir
import concourse.tile as tile
from concourse import bacc
from concourse.bass_utils import run_bass_kernel_spmd

P = 128
C = 768
KC = C // P            # 6 contraction chunks
N_CORES = 8
B, H, W = 16, 56, 56
ROWS = B * H * W       # 50176
RPC = ROWS // N_CORES  # 6272 rows per core
TT = RPC // P          # 49 token tiles per core
TBLK = 4               # token tiles per input DMA block (512 tokens)
HBLK = 4               # token tiles in the fused head block
HSUB = 4               # tiles per kc-outer sub-block (psum capacity)
HW_ = C + HBLK * P     # head chunk cols per kc: 768 w + 1024 x0
N_WARM = 16            # PE pre-warm matmuls issued during the DMA head
OUT_BF16 = os.environ.get("GWTA_OUT", "bf16") == "bf16"

# Internal matmul dtype: bf16 halves input DMA and streams 1 col/cycle.
VARIANT = os.environ.get("GWTA_VARIANT", "bf16")

LAST_STATS: dict = {}

_IN_DT = {
    "bf16": mybir.dt.bfloat16,
    "fp32r": mybir.dt.float32r,
    "fp32": mybir.dt.float32,
}


def _build_nc(variant: str) -> bass.Bass:
    in_dt = _IN_DT[variant]
    out_dt = mybir.dt.bfloat16 if OUT_BF16 else mybir.dt.float32
    nc = bacc.Bacc(None, target_bir_lowering=False)
    hx = nc.declare_dram_parameter("hx", [P, KC, HW_], in_dt, isOutput=False)
    xT = nc.declare_dram_parameter("xT", [C, RPC], in_dt, isOutput=False)
    b = nc.declare_dram_parameter("b", [P, C], mybir.dt.bfloat16, isOutput=False)
    out = nc.declare_dram_parameter("out", [RPC, C], out_dt, isOutput=True)

    with tile.TileContext(nc) as tc:
        with (
            tc.tile_pool(name="const", bufs=1) as const,
            tc.tile_pool(name="xp", bufs=3) as xp,
            tc.tile_pool(name="op", bufs=8) as op,
            tc.tile_pool(name="pp", bufs=1, space="PSUM") as pp,
        ):
            # PE pre-warm: matmuls on zeroed SBUF keep the PE busy from the
            # end of the NEFF preamble (~6.5us) until the first head chunk
            # lands (~9.7us), so the HAM activity window accumulates
            # continuous busy time and un-throttles to 2.4GHz shortly after
            # the real stream begins.  The memset is small so the first
            # warm matmul issues as early as possible.
            g_rhs = const.tile([P, 256], in_dt)
            nc.vector.memset(g_rhs[:], 0.0)
            warm = pp.tile([P, C], mybir.dt.float32, tag="pt3")
            for _ in range(N_WARM):
                nc.tensor.matmul(
                    warm[:, 0:256], g_rhs[:, 0:P], g_rhs[:],
                    start=True, stop=True,
                )

            xTr = xT.rearrange("(kc p) t -> p kc t", p=P)

            # Head chunks: [w_kc | x0_kc] fused, one DMA per kc on the sync
            # ring.  Pass kc of block 0 needs only chunk kc.  Chunk 0 is
            # split so the very first matmul is gated by just w0 + one x
            # tile (229KB) instead of the full chunk.
            # Chunks alternate between the two HWDGE rings: a single ring
            # sustains only ~280GB/s because each DMA's completion receipt
            # (~1-2us) bubbles its queue; the SDMA engines round-robin
            # across queues at packet granularity, so two rings pipeline
            # one chunk's receipt under the other's transfer.
            ht = const.tile([P, KC, HW_], in_dt, name="ht")
            nc.sync.dma_start(out=ht[:, 0, 0 : C + P], in_=hx[:, 0, 0 : C + P])
            nc.scalar.dma_start(out=ht[:, 0, C + P :], in_=hx[:, 0, C + P :])
            for kc in range(1, KC):
                eng = nc.sync if kc % 2 else nc.scalar
                eng.dma_start(out=ht[:, kc, :], in_=hx[:, kc, :])
            # Ramp: tiny blocks right after the head so each deadline
            # trails its DMA arrival, a 1-tile final block for a short tail.
            blocks = [1, 2] + [TBLK] * 10 + [1, 1]
            assert HBLK + sum(blocks) == TT

            xbts: dict = {}

            def issue_xb(j: int) -> None:
                nbj = blocks[j]
                t0j = HBLK + sum(blocks[:j])
                xt = xp.tile([P, KC, TBLK * P], in_dt, tag="xb", name="xb")
                nc.sync.dma_start(
                    out=xt[:, :, : nbj * P],
                    in_=xTr[:, :, t0j * P : (t0j + nbj) * P],
                )
                xbts[j] = xt

            # The bias (bf16) gates block 0's psum evictions, whose tags
            # block 1's later tiles reuse.  It follows the head chunks on
            # the scalar ring, well before the first output (~20us).
            bt = const.tile([P, C], mybir.dt.bfloat16)
            nc.scalar.dma_start(out=bt[:], in_=b[:])
            issue_xb(0)
            issue_xb(1)

            def wslice(kc, half):
                return (
                    ht[:, kc, 0:512] if half == 0 else ht[:, kc, 512:C]
                )

            def evict_last(pt) -> None:
                # Final tile: per-bank TT halves, each store issued the
                # moment its half is ready, split across both HWDGE rings.
                # The kc=5 matmul emission order below makes the [512:C]
                # bank finish FIRST, so its TT and store run while the PE
                # streams the last 512-col matmul.
                g = TT - 1
                ot = op.tile([P, C], out_dt, tag="ot")
                nc.vector.tensor_add(
                    out=ot[:, 512:C], in0=pt[:, 512:C], in1=bt[:, 512:C]
                )
                nc.sync.dma_start(
                    out=out[g * P : (g + 1) * P, 512:C], in_=ot[:, 512:C]
                )
                nc.vector.tensor_add(
                    out=ot[:, 0:512], in0=pt[:, 0:512], in1=bt[:, 0:512]
                )
                nc.scalar.dma_start(
                    out=out[g * P : (g + 1) * P, 0:512], in_=ot[:, 0:512]
                )

            def evict(g: int, pt) -> None:
                ot = op.tile([P, C], out_dt, tag="ot")
                # One fused TT per tile: DVE reads may span psum banks
                # (only matmul WRITES are bank-limited), it is ~25% faster
                # than a 512+256 pair, and it keeps each eviction atomic --
                # the scheduler otherwise interleaves pair halves across
                # tiles, delaying the psum-tag release the next block waits
                # on.
                nc.vector.tensor_add(
                    out=ot[:], in0=pt[:], in1=bt[:]
                )
                nc.scalar.dma_start(
                    out=out[g * P : (g + 1) * P, :], in_=ot[:]
                )

            # Block 0: two kc-outer sub-blocks of HSUB tiles -- pass kc
            # only needs head chunk kc, so the PE rides the chunk arrival
            # curve from the first one.  Each kc=5 pass runs s-ascending,
            # so psum tag g%4 is cleared in the order the next sub-block
            # reuses it.
            for sub in range(HBLK // HSUB):
                pts = [
                    pp.tile(
                        [P, C], mybir.dt.float32,
                        tag=f"pt{(sub * HSUB + s) % 4}",
                        name=f"pt{(sub * HSUB + s) % 4}",
                    )
                    for s in range(HSUB)
                ]
                for kc in range(KC):
                    for s in range(HSUB):
                        g = sub * HSUB + s
                        lhsT = ht[:, kc, C + g * P : C + (g + 1) * P]
                        nc.tensor.matmul(
                            pts[s][:, 0:512], lhsT, wslice(kc, 0),
                            start=(kc == 0), stop=(kc == KC - 1),
                        )
                        nc.tensor.matmul(
                            pts[s][:, 512:C], lhsT, wslice(kc, 1),
                            start=(kc == 0), stop=(kc == KC - 1),
                        )
                for s in range(HSUB):
                    evict(sub * HSUB + s, pts[s])

            # Blocks 1..: s-outer, one 4-tile input DMA each, prefetch
            # distance 2.
            g0 = HBLK
            for bi, nb in enumerate(blocks):
                if bi + 2 < len(blocks):
                    issue_xb(bi + 2)
                xt = xbts.pop(bi)
                # Natural %4 rotation: tile g reuses the tag freed by
                # tile g-4, whose eviction is the EARLIEST completed of the
                # previous four (block 0's kc-outer evictions release in
                # s-ascending order).
                pts = [
                    pp.tile(
                        [P, C], mybir.dt.float32,
                        tag=f"pt{(g0 + s) % 4}", name=f"pt{(g0 + s) % 4}",
                    )
                    for s in range(nb)
                ]
                for s in range(nb):
                    if g0 + s == TT - 1:
                        # Final tile: run the whole [512:C] accumulation
                        # group first, then the [0:512] group.  The narrow
                        # bank then stops ~1.3us before stream end -- its
                        # eviction and store complete during the final
                        # matmuls, and the wide bank's eviction starts the
                        # moment the stream ends instead of queuing behind
                        # it on the DVE.
                        for h in (1, 0):
                            for kc in range(KC):
                                lhsT = xt[:, kc, s * P : (s + 1) * P]
                                nc.tensor.matmul(
                                    pts[s][:, 0:512] if h == 0
                                    else pts[s][:, 512:C],
                                    lhsT, wslice(kc, h),
                                    start=(kc == 0), stop=(kc == KC - 1),
                                )
                        continue
                    for kc in range(KC):
                        lhsT = xt[:, kc, s * P : (s + 1) * P]
                        for h in (0, 1):
                            nc.tensor.matmul(
                                pts[s][:, 0:512] if h == 0
                                else pts[s][:, 512:C],
                                lhsT, wslice(kc, h),
                                start=(kc == 0), stop=(kc == KC - 1),
                            )
                for s in range(nb):
                    if g0 + s == TT - 1:
                        evict_last(pts[s])
                    else:
                        evict(g0 + s, pts[s])
                g0 += nb
    nc.compile()
    return nc


def _fold_weights(qkv_w, qkv_b, proj_w, proj_b, pe):
    v_w = qkv_w[2 * 4 : 3 * 4].astype(np.float64)   # [4, 4]
    v_b = qkv_b[2 * 4 : 3 * 4].astype(np.float64)   # [4]
    bd = np.kron(np.eye(C // 4), v_w.T)             # y_flat @ bd == groupwise v
    w_eff = bd @ proj_w.astype(np.float64).T        # [768, 768]
    b_eff = (
        np.tile(v_b, C // 4) @ proj_w.astype(np.float64).T
        + proj_b.astype(np.float64)
        + pe[:C].astype(np.float64) @ w_eff
    )
    return w_eff, b_eff


def _enable_tracing_shims():
    """Dev-only (GWTA_TRACE=1): restore the NTFF profile hook that this
    image's `antenv` is missing, and keep trace artifacts local instead of
    uploading.  Never active when the kernel is called normally."""
    import sys
    import types

    try:
        from antenv import axon_hooks  # noqa: F401
    except ImportError:
        import antenv
        from trn_agent_boot.trn_boot import _ntff_profile_via_ctypes

        mod = types.ModuleType("antenv.axon_hooks")
        mod._hook = _ntff_profile_via_ctypes("/opt/axon/libaxon_pjrt.so")
        mod.get_axon_ntff_profile_hook = lambda: mod._hook
        mod.set_axon_ntff_profile_hook = lambda h: setattr(mod, "_hook", h)
        sys.modules["antenv.axon_hooks"] = mod
        antenv.axon_hooks = mod

    import concourse.bass_utils as bu

    bu.upload_artifacts = lambda tmpdir: f"local:{tmpdir}"


def kernel(x, qkv_w, qkv_b, proj_w, proj_b, pe):
    x = np.asarray(x, np.float32)
    w_eff, b_eff = _fold_weights(
        np.asarray(qkv_w), np.asarray(qkv_b),
        np.asarray(proj_w), np.asarray(proj_b), np.asarray(pe),
    )

    variant = VARIANT
    if variant == "bf16":
        cast = lambda a: np.ascontiguousarray(a, dtype=ml_dtypes.bfloat16)
    else:
        cast = lambda a: np.ascontiguousarray(a, dtype=np.float32)

    w_dev = cast(w_eff)                       # [768, 768]
    w_chunks = np.asarray(w_dev).reshape(KC, P, C)  # [kc, p, c]
    b_dev = np.broadcast_to(
        b_eff.astype(np.float32).astype(ml_dtypes.bfloat16), (P, C)
    ).copy()

    x_flat = x.reshape(ROWS, C)
    in_maps = []
    for c in range(N_CORES):
        xc = x_flat[c * RPC : (c + 1) * RPC]
        xt = cast(xc.T)                       # [768, 6272]
        xt_np = np.asarray(xt)
        # Fused head: [p, kc, 768 w | 512 x0]
        hx = np.empty((P, KC, HW_), dtype=xt_np.dtype)
        hx[:, :, :C] = w_chunks.transpose(1, 0, 2)
        hx[:, :, C:] = (
            xt_np[:, : HBLK * P].reshape(KC, P, HBLK * P).transpose(1, 0, 2)
        )
        in_maps.append({"hx": hx, "xT": xt_np, "b": b_dev})

    nc = _build_nc(variant)
    trace = bool(int(os.environ.get("GWTA_TRACE", "0")))
    kw = {}
    if trace:
        _enable_tracing_shims()
        kw["tmpdir"] = os.environ.get("GWTA_TRACE_DIR") or None
    r = run_bass_kernel_spmd(nc, in_maps, list(range(N_CORES)), trace=trace, **kw)

    LAST_STATS.clear()
    LAST_STATS.update(
        exec_time_ns=r.exec_time_ns,
        mean_exec_time_ns=r.mean_exec_time_ns,
        variant=variant,
    )

    out = np.empty((ROWS, C), np.float32)
    for c in range(N_CORES):
        out[c * RPC : (c + 1) * RPC] = np.asarray(
            r.results[c]["out"]
        ).astype(np.float32)
    return out.reshape(B, H, W, C)
